# revision 35
# baseline (speedup 1.0000x reference)
"""Chamfer distance (adv->ori direction) Trainium2 Bass kernel.

Problem: adv_pc [8, 4096, 3], ori_pc [8, 4096, 3], weights [8] ->
scalar f32 loss = mean_b( w_b * mean_k( min_j ||adv_bk - ori_bj||^2 ) ).

Sharding: data parallel over the batch dim - core b handles batch b.

Per-core algorithm (K = 4096 points):
  m'[k, j]  = b2_j/2 - a_k . b_j        (augmented matmul, contract dim 4:
                                         ahat = (-a, 1), bhat = (b, b2/2))
  out_core  = sum_k ( a2_k + 2 * min_j m'[k, j] )     (= 4096 * loss1_b)
a2_k is added per-point BEFORE the sum over k (the min is ~ -1.5 and a2
~ +3.0; summing them separately would lose the small result to
cancellation).

PE work is a 2-term fp16 decomposition: the b side is split exactly
(bhat = bh + bl, both fp16), the a side is rounded to fp16 once:
m' = ah.bh + ah.bl, dropping al.bhat (~2^-11 |a||b|, validated 1.7e-3
worst-case end-to-end). fp16 streams at full rate (fp32 matmul = 2
passes each at half rate = 4x), and 2 passes need no mid-chain
LDWEIGHTS reload.

Operand staging is done on the HOST (numpy, O(K) layout work): the
contract-major hi/lo operand tensors are laid out exactly as the PE
reads them, so the device preamble is a handful of large-segment DMAs.
On-device cross-partition gathers (hundreds of 512B DMA segments) were
measured to hide ~12 us of latency - that was the old preamble cost.

The moving operand is j-SPLIT across the four PE row-group quadrants
(group g = (2q+b)%4 owns two 4-k-tile runs of j), so it needs no
replication; the stationary side is replicated to all 4 quadrants by
re-reading the small DRAM tensor.

The j-min: 128 waves of [128, 1024] PSUM (2 banks) rotate 4-deep
through the 8 banks; per wave ScalarE copies the second bank to SBUF
while VectorE runs a custom fused DVE op (TT_MIN_REDUCE_ANT, registered
at import into dve_ops.OPS: out = min(in0, in1), accum_out = free-dim
min) over the first bank (PSUM port) + the copy (SBUF port), scanning
2 fp32/cycle. The 4-deep rotation keeps the per-buffer serial chain
(matmuls -> copy -> fused reduce) off the critical path; the stock
TENSOR_TENSOR_REDUCE ISA opcode crashes this runtime, hence the
custom-ucode op.
"""

import numpy as np

B = 8
K = 4096
KT = K // 128   # 32 k-tiles of 128 adv points
NWV = 4 * KT    # 128 waves of 1024 j each
NCORES = 8

_NC_CACHE = {}

_TTMINR_NAME = "TT_MIN_REDUCE_ANT"


def _register_tt_min_reduce():
    """Custom DVE op via the per-NEFF extension path (dve_ops.OPS):
    out = min(in0, in1); accum_out = min(s0, min_k out[:, k])."""
    from concourse import dve_ops
    from concourse.dve_spec import Spec, Src0, Src1, C0, minn, AluOp, lower
    from concourse.dve_uop import DveOpSpec

    for op in dve_ops.OPS:
        if op.name == _TTMINR_NAME:
            return op

    def _ref(in0, in1, s0, s1, imm2):
        b = np.minimum(in0.astype(np.float32), in1.astype(np.float32))
        acc = np.minimum(
            np.asarray(s0, np.float32),
            b.reshape(b.shape[0], -1).min(axis=-1, keepdims=True),
        )
        return b, acc

    spec = Spec(body=minn(Src0, Src1), accum=AluOp.MIN, accum_init=C0,
                reference=_ref)
    row = dve_ops._CUSTOM_DVE_ROW_BASE + len(dve_ops.OPS)
    assert row < 0x20, "byte-36 row field overflow"
    shas = {}
    for ver in ("v3", "v4"):
        tmp = DveOpSpec(name=_TTMINR_NAME, opcode=row,
                        uops=lower(spec, ver=ver), rd1_en=True)
        shas[ver] = tmp.sha(ver)
    op = dve_ops.DveOp(_TTMINR_NAME, spec, subdim=False, uops_sha=shas)
    dve_ops.OPS.append(op)
    dve_ops.CUSTOM_DVE_SPECS[_TTMINR_NAME] = spec
    dve_ops._SUB_OPCODE_FOR_NAME[_TTMINR_NAME] = row
    return op


def _build_nc():
    import concourse.bacc as bacc
    import concourse.mybir as mybir
    import concourse.tile as tile

    ttminr = _register_tt_min_reduce()

    f32 = mybir.dt.float32
    f16 = mybir.dt.float16
    Alu = mybir.AluOpType
    Ax = mybir.AxisListType

    nc = bacc.Bacc("TRN2", target_bir_lowering=False, debug=False,
                   num_devices=NCORES)

    # host-staged operands (see _stage_inputs): hla row q = contract row
    # q of fp16(ahat), cols t*128 + p (point 32p+t); hlo row 4g+q =
    # quadrant g's j-share of bhat, cols l*256 + hl*128 + p
    # (local j-tile l of 8, hi|lo fp16).
    hla = nc.dram_tensor("hla", [4, K], f16, kind="ExternalInput").ap()
    hlo = nc.dram_tensor("hlo", [16, 2 * K // 4], f16,
                         kind="ExternalInput").ap()
    adv = nc.dram_tensor("adv", [K, 3], f32, kind="ExternalInput").ap()
    out = nc.dram_tensor("out", [1, 1], f32, kind="ExternalOutput").ap()

    with tile.TileContext(nc) as tc:
        with tc.tile_pool(name="sb", bufs=1) as sb:
            HLa = sb.tile([128, K], f16)
            HLo = sb.tile([128, 2 * K // 4], f16)
            ones_t = sb.tile([128, 1], f32)
            nc.gpsimd.memset(ones_t[:], 1.0)

            # Operand DMAs: per (tensor, quadrant), contiguous multi-KB
            # segments per destination partition. The first few waves'
            # slices (k-tiles 0-3, j-run l<4 of quadrants 0-3) go as
            # small separate DMAs so wave 0 starts ~1.5 us after trigger;
            # the bulk follows, spread over the 3 DMA queues.
            # Early DMAs avoid the Scalar queue: the auto-inserted
            # ACT_TABLE_LOAD for the copies sits at its head (~1.3 us).
            qse = (nc.sync, nc.gpsimd)
            for g in range(4):  # early: all of HLo (small), HLa k-tiles 0-3
                qse[g % 2].dma_start(out=HLo[32 * g:32 * g + 4, :],
                                     in_=hlo[4 * g:4 * g + 4, :])
                qse[(g + 1) % 2].dma_start(out=HLa[32 * g:32 * g + 4, 0:512],
                                           in_=hla[:, 0:512])
            qs = (nc.sync, nc.scalar, nc.gpsimd)
            for g in range(4):  # bulk of HLa
                qs[g % 3].dma_start(out=HLa[32 * g:32 * g + 4, 512:],
                                    in_=hla[:, 512:])

            # a2 per adv point on-device: Pa row p = points 32p..32p+31
            # as xyz triples; a2arr[p, t] = ||point 32p+t||^2.
            Pa = sb.tile([128, 3 * KT], f32)
            Asq = sb.tile([128, 3 * KT], f32)
            a2arr = sb.tile([128, KT], f32)
            nc.sync.dma_start(
                out=Pa[:], in_=adv.rearrange("(p c) d -> p (c d)", p=128))
            nc.vector.tensor_tensor(Asq[:], Pa[:], Pa[:], op=Alu.mult)
            Asq_v = Asq[:].rearrange("p (n d) -> p n d", d=3)
            nc.vector.tensor_reduce(a2arr[:], Asq_v, axis=Ax.X, op=Alu.add)

            # Main loop: 128 waves of [128, 1024] PSUM (2 banks) rotating
            # 4-deep. Wave w: k-tile t=w//4, j-quarter q=w%4; bank b is
            # filled by row group g=(2q+b)%4 with 2-pass fp16 matmuls
            # (copy-source bank first); ScalarE copies bank 1 to SBUF and
            # the custom DVE op min-reduces bank 0 (PSUM) + copy (SBUF).
            gminP = sb.tile([128, NWV], f32)
            with tc.tile_pool(name="mm", bufs=4, space="PSUM") as mm, \
                 tc.tile_pool(name="cp", bufs=3) as cp:
                for w in range(NWV):
                    t, q = divmod(w, 4)
                    ps = mm.tile([128, 1024], f32, tag="ps")
                    ops = []
                    for bank in (1, 0):  # copy-source bank first
                        g = (2 * q + bank) % 4
                        r = 32 * g
                        l0 = 4 * (q // 2)  # local j-tile run in HLo
                        a_op = HLa[r:r + 4, t * 128:(t + 1) * 128]
                        bv = HLo[r:r + 4, :].rearrange(
                            "q (l hl p) -> q l hl p", hl=2, p=128)
                        b_hi = bv[:, l0:l0 + 4, 0, :]
                        b_lo = bv[:, l0:l0 + 4, 1, :]
                        o = ps[:, bank * 512:(bank + 1) * 512]
                        ops.append((o, a_op, b_hi, b_lo, r))
                    # pass-major emission: both quadrants' passes
                    # back-to-back; one LDWEIGHTS per quadrant per wave.
                    for o, a_op, b_hi, b_lo, r in ops:
                        nc.tensor.matmul(o, a_op, b_hi, start=True,
                                         stop=False, tile_position=(r, 0))
                    for o, a_op, b_hi, b_lo, r in ops:
                        nc.tensor.matmul(o, a_op, b_lo, start=False,
                                         stop=True, tile_position=(r, 0))
                    cpb = cp.tile([128, 512], f32, tag="cpb")
                    tout = cp.tile([128, 512], f32, tag="tout")
                    nc.scalar.copy(cpb[:], ps[:, 512:1024])
                    nc.vector._custom_dve(
                        ttminr, out=tout[:], in0=ps[:, 0:512], in1=cpb[:],
                        s0=3.0e38, accum_out=gminP[:, w:w + 1])

                # Combine: min over the four waves per k-tile, then
                # 2*min + a2 per point, sum over points, partition-sum
                # via matmul-with-ones (a [1,1] out DMA is 1 segment; a
                # [128,1] out would be 128 tiny segments ~6us hidden).
                gmin2 = sb.tile([128, KT], f32)
                tot = sb.tile([128, KT], f32)
                ksum = sb.tile([128, 1], f32)
                res = sb.tile([1, 1], f32)
                gminP_v = gminP[:].rearrange("p (t h) -> p t h", h=4)
                nc.vector.tensor_reduce(gmin2[:], gminP_v, axis=Ax.X,
                                        op=Alu.min)
                nc.vector.scalar_tensor_tensor(
                    out=tot[:], in0=gmin2[:], scalar=2.0, in1=a2arr[:],
                    op0=Alu.mult, op1=Alu.add)
                nc.vector.tensor_reduce(ksum[:], tot[:], axis=Ax.X,
                                        op=Alu.add)
                ps = mm.tile([128, 1024], f32, tag="ps")
                nc.tensor.matmul(ps[:1, :1], ksum[:], ones_t[:],
                                 start=True, stop=True)
                nc.vector.tensor_copy(res[:], ps[:1, :1])
                nc.sync.dma_start(out=out[:], in_=res[:])

    nc.compile()
    return nc


def _get_nc():
    if "nc" not in _NC_CACHE:
        _NC_CACHE["nc"] = _build_nc()
    return _NC_CACHE["nc"]





def _stage_inputs(adv_b, ori_b):
    """Host-side O(K) operand layout for one batch/core.

    hla [4, 4096] fp16: row q = contract row q of fp16(ahat), ahat =
      (-a, 1); col t*128 + p = point 32p+t.
    hlo [16, 2048] fp16: row 4g+q = contract row q of bhat = (b, b2/2)
      restricted to quadrant g's j-share (j-tile runs
      {(g%2)*4 + (g//2)*8 + 16*m + i : m in 0..1, i in 0..3});
      col l*256 + hl*128 + p = local j-tile l (0..7), point 32p+jt(l).
    """
    a = adv_b.astype(np.float32)
    o = ori_b.astype(np.float32)
    o2 = (o * o).sum(-1)
    ahat = np.concatenate([-a, np.ones((K, 1), np.float32)], 1).T  # [4, K]
    bhat = np.concatenate([o, (o2 / 2)[:, None]], 1).T             # [4, K]

    def hilo_layout(x, tiles):
        # x [4, K] fp32; tiles: list of k-tile indices in layout order.
        hi = x.astype(np.float16)
        lo = (x - hi.astype(np.float32)).astype(np.float16)
        # point 32p+t -> [4, hl, p, t]
        out = np.empty((4, len(tiles), 2, 128), dtype=np.float16)
        hi_v = hi.reshape(4, 128, 32)   # [q, p, t]
        lo_v = lo.reshape(4, 128, 32)
        for li, t in enumerate(tiles):
            out[:, li, 0, :] = hi_v[:, :, t]
            out[:, li, 1, :] = lo_v[:, :, t]
        return out

    # a side: fp16 hi only, col t*128 + p = point 32p+t
    hla = np.ascontiguousarray(
        ahat.astype(np.float16).reshape(4, 128, 32).transpose(0, 2, 1)
    ).reshape(4, K)

    hlo = np.empty((16, 2 * K // 4), dtype=np.float16)
    for g in range(4):
        tiles = [(g % 2) * 4 + (g // 2) * 8 + 16 * m + i
                 for m in range(2) for i in range(4)]
        hlo[4 * g:4 * g + 4, :] = hilo_layout(bhat, tiles).reshape(4, -1)
    return {"hla": hla, "hlo": hlo, "adv": np.ascontiguousarray(adv_b)}


def kernel(adv_pc, ori_pc, weights):
    from concourse.bass_utils import run_bass_kernel_spmd

    adv_pc = np.asarray(adv_pc, dtype=np.float32)
    ori_pc = np.asarray(ori_pc, dtype=np.float32)
    weights = np.asarray(weights, dtype=np.float32)

    nc = _get_nc()
    in_maps = [_stage_inputs(adv_pc[b], ori_pc[b]) for b in range(B)]
    res = run_bass_kernel_spmd(nc, in_maps, core_ids=list(range(NCORES)))
    sums = np.array([res.results[b]["out"][0, 0] for b in range(B)],
                    dtype=np.float32)
    loss1 = sums / np.float32(K)
    return np.array(np.mean(loss1 * weights), dtype=np.float32)


if __name__ == "__main__":
    rng = np.random.default_rng(0)
    a = rng.standard_normal((B, K, 3), dtype=np.float32)
    o = rng.standard_normal((B, K, 3), dtype=np.float32)
    w = np.ones((B,), dtype=np.float32)
    print(kernel(a, o, w))
